# revision 12
# baseline (speedup 1.0000x reference)
"""Trainium2 Bass kernel for nn_IntraClassLoss (segment_reduce) — v2.

Math: inputs [B,C,H,W] logits, targets [B,H,W] int labels, C=4.
probs = softmax(inputs, axis=1); for classes c in 1..C-1:
  cnt_c = #pixels with target==c
  S1_c  = sum over those pixels of p_c
  S2_c  = sum over those pixels of p_c^2
  mean_c = S1_c/(cnt_c+eps); var_c = (S2_c - 2*mean_c*S1_c + cnt_c*mean_c^2)/(cnt_c+eps)
  loss = sum_{c: cnt_c>0} var_c / (C-1)

Sharding: data-parallel over batch, 2 batches per core on 8 cores. Each core
reduces its shard to per-class partials which are DMA'd out and finished on
the host (no collectives needed).

Engine balance (per [128, n] chunk; DMA ~360 GB/s is the roofline):
  ACT : e_c = exp(x_c) -> bf16 (4 ops); S2 for classes 1,2 via Square+accum
  PE  : denom = sum_c e_c via identity-matmul accumulation into PSUM halves;
        S2 for class 3 via ones-vector matmul rows accumulated in PSUM
  DVE : r32 = 1/denom (reciprocal_approx_fast, per PSUM half);
        p_1 = e_1*rbf; pc_c + S1_c via scalar_tensor_tensor accum (is_equal,
        mult) -- select and reduction fused in one op; sq3 = pc_3*pc_3
  Pool: rbf = bf16(r32) cast; p_2, p_3 = e_{2,3}*rbf
The final chunk of each batch half is split into 512-wide subchunks to
shorten the serial tail after the last DMA.
"""

import numpy as np
import ml_dtypes

import concourse.bass as bass
import concourse.bacc as bacc
import concourse.tile as tile
from concourse import mybir
from concourse.bass_utils import run_bass_kernel_spmd

F32 = mybir.dt.float32
BF16 = mybir.dt.bfloat16
I32 = mybir.dt.int32
AF = mybir.ActivationFunctionType
ALU = mybir.AluOpType

B, C, H, W = 16, 4, 1024, 1024
N_CORES = 8
B_LOC = B // N_CORES
P = 128
EPS = 1e-6


def _chunk_plan(b_loc, free, n_chunk, tail_split):
    """Per-core chunk list: (batch, col_offset, width). The final n_chunk of
    the last batch is split into `tail_split` equal subchunks."""
    chunks = []
    per_batch = free // n_chunk
    for b in range(b_loc):
        for k in range(per_batch):
            last = (b == b_loc - 1) and (k == per_batch - 1)
            off = k * n_chunk
            if last and tail_split > 1:
                w = n_chunk // tail_split
                for s in range(tail_split):
                    chunks.append((b, off + s * w, w))
            else:
                chunks.append((b, off, n_chunk))
    return chunks


def build_program(b_loc=B_LOC, h=H, w=W, n_chunk=2048, tail_split=1,
                  den_c_outer=True, s2_mode="act3", delay=2,
                  e_bufs=3, x_bufs=4, p3_dve=True, junk_bufs=1,
                  rbf_bufs=2, r32_bufs=2, pc_bufs=2, p_bufs=3, t_bufs=3,
                  sq_split=1, pc_inplace=False, pair_dma=False, t_on_pool=False,
                  last_fresh=False, recip_psum=False, den_bufs=2,
                  last_split=0, last_rows=0, unpair_last=False,
                  last_p_dve=False, tail_prio=0, last_xsplit=2,
                  p1_pool_steady=False, presplit_n2=False, late_t=0):
    plane = h * w
    free = plane // P
    chunks = _chunk_plan(b_loc, free, n_chunk, tail_split)
    n_cols = len(chunks)

    nc = bacc.Bacc("TRN2", target_bir_lowering=False, debug=False)

    inputs_d = nc.dram_tensor("inputs", [b_loc, C, h, w], F32, kind="ExternalInput")
    targets_d = nc.dram_tensor("targets", [b_loc, h, w], I32, kind="ExternalInput")
    ident_d = nc.dram_tensor("ident", [P, P], BF16, kind="ExternalInput")
    ones_d = nc.dram_tensor("ones", [P, 1], BF16, kind="ExternalInput")
    # s2_mode: 'act3' (3 ACT sq+accum cols), 'act2_row1' (2 ACT + 1 PE row),
    #          'rows3' (3 squares on DVE/Pool + 3 PE rows)
    n_s2 = {"act3": 3, "act2_row1": 2, "rows3": 0}[s2_mode]
    n_rows = 3 - n_s2
    n_lrow = 3 if last_rows else 0
    # out: [3+n_s2, 128, n_cols] = s1_1..3 then s2 cols
    n_cols_out = n_cols * (2 if last_split else sq_split)
    out_d = nc.dram_tensor(
        "out", [P, (3 + n_s2) * n_cols_out], F32, kind="ExternalOutput"
    )
    # out row: s2_3 partial row [1, 512]
    row_d = (
        nc.dram_tensor("rowout", [n_rows + n_lrow, 512], F32, kind="ExternalOutput")
        if (n_rows + n_lrow)
        else None
    )

    with tile.TileContext(nc) as tc:
        with (
            tc.tile_pool(name="const", bufs=1) as constp,
            tc.tile_pool(name="io", bufs=x_bufs) as iop,
            tc.tile_pool(name="tio", bufs=t_bufs) as tiop,
            tc.tile_pool(name="pp", bufs=p_bufs) as pp,
            tc.tile_pool(name="ep", bufs=e_bufs) as ep,
            tc.tile_pool(name="rp32", bufs=r32_bufs) as rp32,
            tc.tile_pool(name="rpbf", bufs=rbf_bufs) as rpbf,
            tc.tile_pool(name="pcp", bufs=pc_bufs) as pcp,
            tc.tile_pool(name="junkp", bufs=junk_bufs) as junkp,
            tc.tile_pool(name="work", bufs=2) as workp,
            tc.tile_pool(name="stats", bufs=1) as statp,
            tc.tile_pool(name="psum", bufs=den_bufs, space="PSUM") as psump,
            tc.tile_pool(name="psrow", bufs=1, space="PSUM") as psrowp,
        ):
            ident = constp.tile([P, P], BF16)
            ones = constp.tile([P, 1], BF16)
            const_dma = [False]

            def emit_const_dmas():
                if not const_dma[0]:
                    nc.sync.dma_start(ident[:], ident_d.ap())
                    nc.sync.dma_start(ones[:], ones_d.ap())
                    const_dma[0] = True

            stat_w = n_cols * (2 if last_split else sq_split)
            stats_all = statp.tile(
                [P, (3 + n_s2) * stat_w], F32, tag="stats", name="stats_all"
            )
            s1_t = [
                stats_all[:, ci * stat_w : (ci + 1) * stat_w] for ci in range(3)
            ]
            s2_t = [
                stats_all[:, (3 + ci) * stat_w : (4 + ci) * stat_w]
                for ci in range(n_s2)
            ]
            if last_split:
                nc.gpsimd.memset(stats_all[:], 0.0)
            s2rows = [
                psrowp.tile([1, 512], F32, tag=f"s2row{k}", name=f"s2row{k}")
                for k in range(n_rows + n_lrow)
            ]

            n_mm_total = sum(max(1, cw // 512) for (_, _, cw) in chunks)
            n_mm_last = sum(
                max(1, cw // 512) for (_, _, cw) in chunks[len(chunks) - last_rows :]
            ) if last_rows else 0
            mm_idx = [0] * (n_rows + n_lrow)

            # Software-pipelined emission: stage F (front: DMA, exp, den,
            # recip, rbf, p-muls) for chunk j is emitted alongside stage T
            # (tail: stt+S1, squares+S2) for chunk j-1, so each in-order
            # engine stream flows without cross-stage stalls.
            front = {}  # j -> (t_tile, ps)
            deferred_t = []

            def qs2(qs):
                return (slice(None), qs)

            def emit_front(j, sq_parts=()):
                b, off, cw = chunks[j]
                taper = last_fresh and cw < n_chunk
                L = "_L" if taper else ""
                Lc = "_L" if (taper and last_fresh == "all") else ""
                sl = slice(off, off + cw)
                es = []
                sq_parts = list(sq_parts)
                if pair_dma:
                    gsz = 4 if pair_dma == 4 else 2
                    pair_ap = inputs_d.ap()[b].rearrange(
                        "c (p a) w -> p c (a w)", p=P
                    )
                    singles = unpair_last and j == len(chunks) - 1
                    views = []
                    if singles:
                        # last chunk: one class per DMA so the denominator can
                        # pre-accumulate classes 0..2 before the final arrival;
                        # the final class arrives as two half transfers so the
                        # first reciprocal half starts before the conveyor ends
                        for c in range(C):
                            xg = iop.tile([P, gsz, cw], F32,
                                          tag=f"xp{c % 2}{L}", name=f"xpg{c}")
                            if c == C - 1:
                                hwc = cw // last_xsplit
                                for q in range(last_xsplit):
                                    qs = slice(q * hwc, (q + 1) * hwc)
                                    nc.sync.dma_start(
                                        xg[:, 0, qs],
                                        pair_ap[:, c : c + 1, off + q * hwc
                                                : off + (q + 1) * hwc],
                                    )
                            else:
                                nc.sync.dma_start(
                                    xg[:, 0, :], pair_ap[:, c : c + 1, sl]
                                )
                            views.append(xg[:, 0, :])
                    else:
                        pres = (presplit_n2 and j != len(chunks) - 1
                                and j >= len(chunks) - 1 - presplit_n2)
                        for g in range(C // gsz):
                            xg = iop.tile([P, gsz, cw], F32, tag=f"xp{g}{L}",
                                          name=f"xpg{g}", bufs=1 if L else None)
                            if pres and g == (C // gsz) - 1:
                                hwc = cw // 2
                                for k in range(gsz):
                                    for q in range(2):
                                        nc.sync.dma_start(
                                            xg[:, k, q * hwc : (q + 1) * hwc],
                                            pair_ap[:, gsz * g + k : gsz * g + k + 1,
                                                    off + q * hwc
                                                    : off + (q + 1) * hwc],
                                        )
                            else:
                                nc.sync.dma_start(
                                    xg[:], pair_ap[:, gsz * g : gsz * (g + 1), sl]
                                )
                            for k in range(gsz):
                                views.append(xg[:, k, :])
                    for c in range(C):
                        e = ep.tile([P, cw], BF16, tag=f"e{c}{Lc}", name=f"ee{c}",
                                    bufs=1 if Lc else None)
                        if singles and c == C - 1:
                            hwc = cw // last_xsplit
                            for q in range(last_xsplit):
                                qs = slice(q * hwc, (q + 1) * hwc)
                                nc.scalar.activation(
                                    e[:, qs], views[c][:, qs], AF.Exp
                                )
                        elif (presplit_n2 and j != len(chunks) - 1
                              and j >= len(chunks) - 1 - presplit_n2
                              and c >= C - gsz):
                            hwc = cw // 2
                            for q in range(2):
                                qs = slice(q * hwc, (q + 1) * hwc)
                                nc.scalar.activation(
                                    e[:, qs], views[c][:, qs], AF.Exp
                                )
                        else:
                            nc.scalar.activation(e[:], views[c], AF.Exp)
                        es.append(e)
                        if c < C - 1:
                            if sq_parts:
                                sq_parts.pop(0)()
                        else:
                            for part in sq_parts:
                                part()
                            sq_parts = []
                else:
                    for c in range(C):
                        x = iop.tile([P, cw], F32, tag="x")
                        x_ap = inputs_d.ap()[b, c].rearrange("(p a) w -> p (a w)", p=P)
                        nc.sync.dma_start(x[:], x_ap[:, sl])
                        e = ep.tile([P, cw], BF16, tag=f"e{c}")
                        nc.scalar.activation(e[:], x[:], AF.Exp)
                        es.append(e)
                        if c < C - 1:
                            if sq_parts:
                                sq_parts.pop(0)()
                        else:
                            for part in sq_parts:
                                part()
                            sq_parts = []
                t_tile = tiop.tile([P, cw], I32, tag=f"t{L}", name="t_tile",
                                   bufs=1 if L else None)
                tgt_ap = targets_d.ap()[b].rearrange("(p a) w -> p (a w)", p=P)
                if late_t and j >= len(chunks) - late_t:
                    # deferred targets go on the in-order SP queue so they
                    # transfer strictly after every x DMA
                    deferred_t.append(
                        lambda t_tile=t_tile, tgt_ap=tgt_ap, sl=sl:
                        nc.sync.dma_start(t_tile[:], tgt_ap[:, sl])
                    )
                else:
                    (nc.gpsimd if t_on_pool else nc.sync).dma_start(
                        t_tile[:], tgt_ap[:, sl]
                    )

                emit_const_dmas()
                nhalf = max(1, cw // 1024)
                hw_ = cw // nhalf
                r32 = (
                    None
                    if recip_psum
                    else rp32.tile([P, cw], F32, tag=f"r32{Lc}", name="r32",
                                   bufs=1 if Lc else None)
                )
                rbf = rpbf.tile([P, cw], BF16, tag=f"rbf{Lc}", name="rbf",
                                bufs=1 if Lc else None)
                for hh in range(nhalf):
                    den = psump.tile([P, hw_], F32, tag="den")
                    base = hh * hw_
                    nslice = max(1, hw_ // 512)
                    for c in range(C):
                        for s in range(nslice):
                            s0 = base + s * 512
                            s1 = min(base + (s + 1) * 512, base + hw_)
                            d_sl = slice(s0 - base, s1 - base)
                            nc.tensor.matmul(
                                den[:, d_sl], ident[:], es[c][:, s0:s1],
                                start=(c == 0), stop=(c == C - 1),
                            )
                    if recip_psum == "sbufh":
                        r32h = rp32.tile([P, hw_], F32, tag="r32h", name="r32h")
                        nc.vector.reciprocal_approx_fast(r32h[:], den[:])
                        nc.vector.tensor_copy(rbf[:, base : base + hw_], r32h[:])
                    elif recip_psum:
                        nc.vector.reciprocal_approx_fast(den[:], den[:])
                        nc.vector.tensor_copy(rbf[:, base : base + hw_], den[:])
                    else:
                        nc.vector.reciprocal_approx_fast(
                            r32[:, base : base + hw_], den[:]
                        )

                if not recip_psum:
                    nc.gpsimd.tensor_copy(rbf[:], r32[:])

                ps = []
                nparts = 2 if (last_split and j >= len(chunks) - last_split
                               and cw > 1024) else 1
                for ci, c in enumerate((1, 2, 3)):
                    p = pp.tile([P, cw], BF16, tag=f"p{ci}{Lc}", name=f"p{ci}",
                                bufs=2 if Lc else None)
                    on_dve = ci == 0 or (ci == 2 and p3_dve) or (
                        last_p_dve and j == len(chunks) - 1
                    )
                    if (p1_pool_steady and ci == 0
                            and j < len(chunks) - 2):
                        on_dve = False
                    if last_p_dve == "n2" and ci == 1 and j == len(chunks) - 2:
                        on_dve = True
                    for q in range(nparts):
                        qs = slice(q * cw // nparts, (q + 1) * cw // nparts)
                        if on_dve:
                            nc.vector.tensor_mul(p[qs2(qs)], es[c][qs2(qs)], rbf[qs2(qs)])
                        else:
                            nc.gpsimd.tensor_mul(p[qs2(qs)], es[c][qs2(qs)], rbf[qs2(qs)])
                    ps.append(p)
                front[j] = (t_tile, ps)

            tail_pcs = {}

            def emit_tail_stt(j):
                _, _, cw = chunks[j]
                t_tile, ps = front.pop(j)
                pcs = [None, None, None]
                nparts = 2 if (last_split and j >= len(chunks) - last_split
                               and cw > 1024) else 1
                # chunk n-2: emit the Pool-gated class-2 select last so DVE
                # isn't idle waiting on Pool's p2
                order = (0, 2, 1) if j == len(chunks) - 2 else (0, 1, 2)
                for ci in order:
                    c = ci + 1
                    pc = ps[ci] if pc_inplace else pcp.tile([P, cw], BF16, tag=f"pc{ci}")
                    for q in range(nparts):
                        qs = slice(q * cw // nparts, (q + 1) * cw // nparts)
                        nc.vector.scalar_tensor_tensor(
                            out=pc[qs2(qs)], in0=t_tile[qs2(qs)], scalar=c,
                            in1=ps[ci][qs2(qs)],
                            op0=ALU.is_equal, op1=ALU.mult,
                            accum_out=s1_t[ci][:, (2 * j + q if last_split else j) : (2 * j + q if last_split else j) + 1],
                        )
                    pcs[ci] = pc
                tail_pcs[j] = pcs

            def _sq_parts(j):
                # yield thunks: each emits one ACT half-square (or PE-row
                # square group) of chunk j; caller interleaves them.
                _, _, cw = chunks[j]
                pcs = tail_pcs[j]
                if last_rows and j >= len(chunks) - last_rows:
                    def lrows_part():
                        for k in range(3):
                            sq = junkp.tile([P, cw], BF16, tag=f"lsq{k}", name=f"lsq{k}")
                            nc.vector.tensor_mul(sq[:], pcs[k][:], pcs[k][:])
                            for s in range(max(1, cw // 512)):
                                w_sl = slice(s * 512, min((s + 1) * 512, cw))
                                cols = w_sl.stop - w_sl.start
                                ridx = n_rows + k
                                nc.tensor.matmul(
                                    s2rows[ridx][:, :cols], ones[:], sq[:, w_sl],
                                    start=(mm_idx[ridx] == 0),
                                    stop=(mm_idx[ridx] == n_mm_last - 1),
                                )
                                mm_idx[ridx] += 1
                    yield lrows_part
                    return
                nsq = sq_split if cw > 512 else 1
                if last_split and j >= len(chunks) - last_split and cw > 1024:
                    nsq = 2
                for ci in range(n_s2):
                    junk = junkp.tile([P, cw], BF16, tag="junk", name="junk")
                    for q in range(nsq):
                        def part(ci=ci, q=q, junk=junk, nsq=nsq):
                            qs = slice(q * cw // nsq, (q + 1) * cw // nsq)
                            nc.scalar.activation(
                                junk[:, qs], pcs[ci][:, qs], AF.Square,
                                accum_out=s2_t[ci][:, (2 * j + q if last_split else j * nsq + q) : (2 * j + q if last_split else j * nsq + q) + 1],
                            )
                        yield part

                def rows_part():
                    for k in range(n_rows):
                        ci = n_s2 + k
                        sq = junkp.tile([P, cw], BF16, tag=f"sq{k}", name=f"sq{k}")
                        if ci == 2:
                            nc.gpsimd.tensor_mul(sq[:], pcs[ci][:], pcs[ci][:])
                        else:
                            nc.vector.tensor_mul(sq[:], pcs[ci][:], pcs[ci][:])
                        for s in range(max(1, cw // 512)):
                            w_sl = slice(s * 512, min((s + 1) * 512, cw))
                            cols = w_sl.stop - w_sl.start
                            nc.tensor.matmul(
                                s2rows[k][:, :cols], ones[:], sq[:, w_sl],
                                start=(mm_idx[k] == 0),
                                stop=(mm_idx[k] == n_mm_total - 1),
                            )
                            mm_idx[k] += 1
                if n_rows:
                    yield rows_part

            def emit_interleaved():
                # 3-stage software pipeline per iteration j:
                #   PE rows of chunk j-delay-1 (deps long ready, keeps PE
                #   stream clean before den(j)), then stt+squares of chunk
                #   j-delay, then front of chunk j. ACT half-squares are
                #   interleaved between the exps of front(j).
                n = len(chunks)
                pending_rows = {}
                import contextlib
                for j in range(n + delay + 1):
                    jr = j - delay - 1
                    if jr in pending_rows:
                        pending_rows.pop(jr)()
                    jt = j - delay
                    sq_parts = []
                    def hoist():
                        if tail_prio and jt >= n - 2:
                            return tc.high_priority(offset=tail_prio)
                        return contextlib.nullcontext()

                    if 0 <= jt < n:
                        with hoist():
                            emit_tail_stt(jt)
                            parts = list(_sq_parts(jt))
                            if n_rows:
                                pending_rows[jt] = parts.pop()
                            sq_parts = parts
                    if j < n:
                        emit_front(j, sq_parts)
                        if j == n - 1:
                            for fire in deferred_t:
                                fire()
                            deferred_t.clear()
                    else:
                        with hoist():
                            for part in sq_parts:
                                part()

            emit_interleaved()

            for k in range(n_rows + n_lrow):
                rowsb = statp.tile([1, 512], F32, tag=f"rowsb{k}", name=f"rowsb{k}")
                nc.vector.tensor_copy(rowsb[:], s2rows[k][:])
                nc.sync.dma_start(row_d.ap()[k], rowsb[:])
            for ci in range(3 + n_s2):
                nc.sync.dma_start(
                    out_d.ap()[:, ci * n_cols_out : (ci + 1) * n_cols_out],
                    stats_all[:, ci * stat_w : (ci + 1) * stat_w],
                )

    nc.compile()
    return nc


_CACHED = {}


BUILD_KW = dict(
    s2_mode="act3", pc_inplace=True, sq_split=1, e_bufs=2,
    t_on_pool=True, pair_dma=2, x_bufs=3, t_bufs=3, p_bufs=3,
    recip_psum="sbufh", rbf_bufs=1, den_bufs=3, unpair_last=True,
    last_xsplit=4, last_split=1, late_t=1,
)


def _get_program():
    if "nc" not in _CACHED:
        _CACHED["nc"] = build_program(**BUILD_KW)
        _CACHED["sq_split"] = BUILD_KW.get("sq_split", 1)
    return _CACHED["nc"]


def finish_host(stats_per_core, rows_per_core, cnt, n_cols, n_s2=3):
    """stats: list of [3+n_s2, 128, n_cols*sq]; rows: list of [n_rows, 512]."""
    s1 = np.zeros(3, dtype=np.float64)
    s2 = np.zeros(3, dtype=np.float64)
    for i, s in enumerate(stats_per_core):
        # s: [128, (3+n_s2)*stat_w] fused stats
        sd = s.astype(np.float64).reshape(128, 3 + n_s2, -1).transpose(1, 0, 2)
        s1 += sd[0:3].sum(axis=(1, 2))
        for ci in range(n_s2):
            s2[ci] += sd[3 + ci].sum()
        if rows_per_core is not None:
            rows = rows_per_core[i]
            nr = 3 - n_s2 if rows.shape[0] in (3 - n_s2,) else rows.shape[0]
            for k in range(3 - n_s2):
                s2[n_s2 + k] += rows[k].astype(np.float64).sum()
            for k in range(rows.shape[0] - (3 - n_s2)):
                s2[k] += rows[(3 - n_s2) + k].astype(np.float64).sum()
    mean = s1 / (cnt + EPS)
    var = (s2 - 2.0 * mean * s1 + cnt * mean * mean) / (cnt + EPS)
    intra = np.where(cnt > 0, var, 0.0).sum()
    return np.float32(intra / (C - 1))


def make_in_maps(inputs, targets):
    ident = np.eye(P, dtype=ml_dtypes.bfloat16)
    ones = np.ones((P, 1), dtype=ml_dtypes.bfloat16)
    return [
        {
            "inputs": np.ascontiguousarray(inputs[i * B_LOC : (i + 1) * B_LOC]),
            "targets": np.ascontiguousarray(targets[i * B_LOC : (i + 1) * B_LOC]),
            "ident": ident,
            "ones": ones,
        }
        for i in range(N_CORES)
    ]


def kernel(inputs: np.ndarray, targets: np.ndarray) -> np.ndarray:
    nc = _get_program()
    in_maps = make_in_maps(inputs, targets)
    res = run_bass_kernel_spmd(nc, in_maps, list(range(N_CORES)))
    stats = [res.results[i]["out"] for i in range(N_CORES)]
    rows = (
        [res.results[i]["rowout"] for i in range(N_CORES)]
        if "rowout" in res.results[0]
        else None
    )
    n_s2 = {"act3": 3, "act2_row1": 2, "rows3": 0}[BUILD_KW.get("s2_mode", "act3")]
    cnt = np.bincount(targets.ravel(), minlength=C)[1:C].astype(np.float64)
    return finish_host(stats, rows, cnt, None, n_s2)
